# revision 50
# baseline (speedup 1.0000x reference)
"""Trainium2 Bass kernel for nn_DihedralGroupConv.

Math: reference computes
    filt[c,i,d,o] = sum_g perm[g,i,o] * weight[g,c,d]
    out = x.reshape(B,-1) @ filt.reshape(C*2n, D*2n)
i.e. out[b,d,o] = sum_{g,c} weight[g,c,d] * x[b,c, idx_g(o)]
where perm[g] are permutation matrices of the dihedral regular
representation: each is a half-wise cyclic shift of either x itself
(rotations) or of the reflected array xr (reflections).

Kernel strategy (data-parallel over batch, 64 b per core):
  - Host precomputes a minimally halo-padded (209 = 4 + 200 + 5) per-half
    image of x, laid out directly as the SBUF image AXW[128, 128+16*418]
    with partition = 32*(b%4) + c (weights packed in the first 128
    columns so they ride the first input DMA chunk), so each generator
    contribution over a quad of 4 batch elements is ONE contiguous-window
    matmul per 32x32 PE tile: rotations read a shifted window forward,
    reflections read a shifted window BACKWARD (negative-stride AP with
    swapped halves).
  - Quads are processed in multi-quad SUPERGROUPS: quad j of a supergroup
    runs on PE tiles (u, (u+j)%4), so up to all 16 32x32 tiles of the
    array stream concurrently (4 quads per 400-cycle sweep).  Supergroup
    sweeps are LDWEIGHTS-issue-bound at the fixed 1.2 GHz NX clock, which
    makes them insensitive to the HAM cold-clock phase -- no PE warm-up
    needed.  All generators accumulate into one PSUM bank per quad
    (8 banks rotated).
  - Schedule shaped for latency at BOTH ends: a 2-quad supergroup first
    (starts as soon as the small first DMA chunk lands), 4-quad
    supergroups in the middle, 2-quad groups + a single + the last quad
    split into two half-width groups at the end, so drains stay ahead
    and the tail after the final matmul is one half-quad cast + one
    small store on an otherwise-idle ring.
  - DRAM tensors are flat 2-D so every DMA is one contiguous run per
    partition (128 descriptors per transfer).  Input chunks and bulk
    stores ride the sync HWDGE ring: its FIFO queues stores BEHIND the
    input chunks, so input keeps full HBM bandwidth until the last chunk
    lands and compute is never starved; the scalar sequencer stays free
    for its ACT drain copies (no DIRECT2D contention).
  - PSUM->SBUF drains (cast to fp16) split across DVE and ACT so they
    keep up with the supergroup matmul cadence.
All DMAs are pure 128-partition contiguous-run transfers; host
unscrambles the [128, 6400] output image (supergroup quads carry a
rotated batch-slot -> partition-block mapping)."""

import numpy as np

import concourse.bass as bass  # noqa: F401  (kept for users of this module)
import concourse.mybir as mybir
from concourse import bacc
from concourse.tile import TileContext
from concourse.bass_utils import run_bass_kernel_spmd

# Problem constants (hardcoded per harness contract).
B = 512
C = 32          # in channels
D = 32          # out channels
N = 200         # half length; 2N = 400
L = 2 * N
N_CORES = 8
BPC = B // N_CORES          # 64 batch per core
NQ = BPC // 4               # 16 quads (4 batch / quad)
HALO_L = 4                  # left halo: rotations need max(-s) = 3
HALO_R = 5                  # right halo: reflection s=0 wrap needs 5
PH = N + HALO_L + HALO_R    # 209 padded half length
QW = 2 * PH                 # elems per quad per partition
WSW = 128                   # weight-image columns at the head of AXW

_DT_IN = mybir.dt.float16   # 1 cyc/col PE mode, 1-pass weight load, half DMA
_DT_OUT = mybir.dt.float16  # output staged/stored as fp16, host casts to f32
_NP_IN = np.float16

_cache = {}


def _derive_gens(perm):
    """Classify each generator as (is_refl, shift s) with y[o] = base[(o+s)%N]
    per half, where base is x (rotation) or xr (reflection)."""
    n = N
    o = np.arange(n)
    gens = []
    for g in range(perm.shape[0]):
        idx = np.argmax(perm[g], axis=0).astype(np.int64)  # y[o] = x[idx[o]]
        # rotation candidate: idx[o] = (o - r) % n ; idx[n+o] = n + (o-r)%n
        r = int((-idx[0]) % n)
        rot = np.concatenate([(o - r) % n, n + (o - r) % n])
        if np.array_equal(idx, rot):
            s = -r if r <= n // 2 else n - r
            gens.append((False, s))
            continue
        # reflection candidate: y[o] = xr[(o+r)%n per half] with
        # xr[t] = x[n + (-t)%n], xr[n+t] = x[(-t)%n]
        # => idx[o] = n + (-o-r)%n ; idx[n+o] = (-o-r)%n
        r = int(idx[0] - n) % n     # idx[0] = n + (-r)%n -> (-r)%n
        r = (-r) % n
        refl = np.concatenate([n + (-o - r) % n, (-o - r) % n])
        if np.array_equal(idx, refl):
            s = r if r <= n // 2 else r - n
            gens.append((True, s))
            continue
        raise NotImplementedError(f"perm[{g}] is not a dihedral rep matrix")
    for is_refl, s in gens:
        if is_refl:
            ok = _refl_j0(s) <= PH - 1
        else:
            ok = -HALO_L <= s <= HALO_R
        if not ok:
            raise NotImplementedError(f"shift {s} exceeds halo")
    return gens


def _refl_j0(s):
    """Start index (per padded half) of a reflection's backward scan:
    position of t = (-s) mod N such that 199 more decreasing indices
    stay inside the padded half."""
    j0 = ((-s) % N) + HALO_L
    if j0 < N - 1:
        j0 += N
    return j0


def _build_program(gens):
    """Build + compile the SPMD Bass program (identical on all cores)."""
    rot = [(j, s) for j, (is_r, s) in enumerate(gens) if not is_r]
    refl = [(j, s) for j, (is_r, s) in enumerate(gens) if is_r]
    nblk = len(rot) + len(refl)
    assert 32 * nblk <= WSW

    nc = bacc.Bacc("TRN2", target_bir_lowering=False, debug=False,
                   num_devices=N_CORES, enable_partition_id=False)
    # Flat 2-D DRAM tensors: one contiguous run per partition per DMA.
    axw_d = nc.dram_tensor("axw", [128, WSW + NQ * QW], _DT_IN,
                           kind="ExternalInput")
    outr_d = nc.dram_tensor("outr", [128, NQ * L], _DT_OUT,
                            kind="ExternalOutput")

    # Compute groups: multi-quad SUPERGROUPS (quad j of the supergroup
    # on PE tiles (u, (u+j)%4): up to all 16 32x32 tiles of the array
    # stream concurrently -- 4 quads per sweep instead of 1).
    # Supergroup sweeps are LDWEIGHTS-issue-bound at the fixed NX clock,
    # which also makes them insensitive to the HAM cold-clock phase --
    # so no PE warm-up is needed at all.  A small 2-quad supergroup
    # first (starts on the small first DMA chunk), a single quad and
    # two half-width groups of the last quad at the end so the kernel
    # tail is one small cast + store.
    groups = [(0, 1), (2, 3, 4, 5), (6, 7, 8, 9), (10, 11), (12, 13),
              (14,)]
    # DMA chunk boundaries in quads, aligned to the groups above
    # (bigger trailing chunk -> better HBM efficiency, fewer DIRECT2Ds,
    # and the store backlog starts draining sooner)
    CHUNKS = [2, 4, 4, 6]
    with TileContext(nc) as tc:
        with (
            tc.tile_pool(name="arrp", bufs=1) as arrp,
            tc.tile_pool(name="wsp", bufs=1) as wsp,
            tc.tile_pool(name="stg", bufs=1) as stgp,
            tc.tile_pool(name="psum", bufs=1, space="PSUM") as psump,
        ):
            axw_sb = arrp.tile([128, WSW + NQ * QW], _DT_IN, name="axw_sb")
            c0 = 0
            for ci, cq in enumerate(CHUNKS):
                a = 0 if ci == 0 else WSW + c0 * QW
                b = WSW + (c0 + cq) * QW
                nc.sync.dma_start(out=axw_sb[:, a:b], in_=axw_d[:, a:b])
                c0 += cq

            # pre-allocated rotating PSUM tiles: one bank per quad in
            # flight, 8 banks ping-ponged (fewer tile instances -> fewer
            # semaphores -> shorter kernel-tail sem-reset storm)
            pstiles = [psump.tile([128, L], mybir.dt.float32,
                                  name=f"ps{i}") for i in range(8)]

            # per quad: one matmul per generator per 32x32 PE quadrant
            # (batch element u -> SBUF rows 32u, PSUM partitions 32u);
            # the 4 quadrants run concurrently in the array.
            # (is_refl, weight block col, window param)
            mm_descs = []
            for k, (_, s) in enumerate(rot):
                mm_descs.append((False, 32 * k, s + HALO_L))
            for k, (_, s) in enumerate(refl):
                mm_descs.append((True, 32 * (len(rot) + k), _refl_j0(s)))
            ng = len(mm_descs)

            axt = axw_sb[:, :]
            pstride = axt.ap[0][0]      # free elems per partition

            def rhs_ap(p0, q, is_r, w, h=None):
                """Matmul window for quad q; h=None -> both rep halves
                (N=400 cols), h in {0,1} -> that rep half only (N=200)."""
                if not is_r:
                    off = p0 * pstride + WSW + q * QW + w
                    if h is None:
                        dims = [[pstride, 32], [PH, 2], [1, N]]
                    else:
                        off += h * PH
                        dims = [[pstride, 32], [1, N]]
                    return bass.AP(axt.tensor, off, dims)
                # reflection: swapped halves, backward o scan from j0=w;
                # out (h, o) reads src[1-h] at t = (-o-s) mod N
                off = p0 * pstride + WSW + q * QW + PH + w
                if h is None:
                    dims = [[pstride, 32], [-PH, 2], [-1, N]]
                else:
                    off -= h * PH
                    dims = [[pstride, 32], [-1, N]]
                return bass.AP(axt.tensor, off, dims)

            stg = stgp.tile([128, NQ, L], _DT_OUT, name="stg")

            # psum bank assignment: rotate through the 8 banks so every
            # bank is freed by its drain a full supergroup ahead of reuse
            psmap = {0: 0, 1: 1,
                     2: 2, 3: 3, 4: 4, 5: 5,
                     6: 6, 7: 7, 8: 0, 9: 1,
                     10: 2, 11: 3, 12: 4, 13: 5,
                     14: 6}
            last_grp = groups[-1]

            for grp in groups:
                pss = [pstiles[psmap[q]] for q in grp]
                for i, (is_r, wc, w) in enumerate(mm_descs):
                    for j, q in enumerate(grp):
                        for u in range(4):
                            p0 = 32 * u
                            # supergroups: quad j uses PE tiles
                            # (u, (u+j)%4) so the quads' matmuls occupy
                            # disjoint PE tiles and stream concurrently
                            v0 = 32 * ((u + j) % 4)
                            nc.tensor.matmul(
                                pss[j][v0:v0 + 32, :],
                                axw_sb[p0:p0 + 32, wc:wc + 32],
                                rhs_ap(p0, q, is_r, w),
                                start=(i == 0), stop=(i == ng - 1),
                                tile_position=(p0, v0),
                            )
                # drains split across DVE and ACT so supergroup drains
                # keep up with the 16-tile matmul cadence.  Bulk stores
                # ride the sync HWDGE ring: its FIFO queues them BEHIND
                # the input chunks, so input keeps full HBM bandwidth
                # until the last chunk lands and the store backlog
                # drains right after -- and the scalar sequencer stays
                # free for the ACT copies (no DIRECT2D contention).
                # The final single quad stores on the scalar ring
                # (empty FIFO -> processes immediately).
                for j, q in enumerate(grp):
                    if j < 2 and grp is not last_grp:
                        nc.vector.tensor_copy(out=stg[:, q],
                                              in_=pss[j][:, :])
                    else:
                        # ACT handles the tail quad's cast so the DVE
                        # chain at the end only carries the q15 halves
                        nc.scalar.copy(out=stg[:, q], in_=pss[j][:, :])
                    if j % 2 == 1 or j == len(grp) - 1:
                        q0 = q - 1 if j % 2 == 1 else q
                        nc.sync.dma_start(
                            out=outr_d[:, q0 * L:(q + 1) * L],
                            in_=stg[:, q0:q + 1, :])

            # final quad, one rep-half at a time; half 0's cast+store
            # (sync ring) overlap half 1's matmuls, half 1 stores on the
            # scalar ring -> no DIRECT2D queueing in the tail
            qf = NQ - 1
            for h in range(2):
                psh = pstiles[7 if h == 0 else 1]   # free banks by now
                for i, (is_r, wc, w) in enumerate(mm_descs):
                    for u in range(4):
                        p0 = 32 * u
                        nc.tensor.matmul(
                            psh[p0:p0 + 32, 0:N],
                            axw_sb[p0:p0 + 32, wc:wc + 32],
                            rhs_ap(p0, qf, is_r, w, h=h),
                            start=(i == 0), stop=(i == ng - 1),
                            tile_position=(p0, p0),
                        )
                nc.vector.tensor_copy(out=stg[:, qf, h * N:(h + 1) * N],
                                      in_=psh[:, 0:N])
            # ONE store for both halves on the scalar ring (empty at this
            # point): the final DIRECT2D issues right after the last cast
            # with no ring queueing, and only one descriptor drain + HBM
            # write-ack is paid in the tail
            nc.scalar.dma_start(out=outr_d[:, qf * L:(qf + 1) * L],
                                in_=stg[:, qf, :])
    nc.compile()
    return nc


def _host_images(x, weight, gens):
    """Build per-core AXW images (weights + halo-padded x)."""
    n = N
    rot = [(j, s) for j, (is_r, s) in enumerate(gens) if not is_r]
    refl = [(j, s) for j, (is_r, s) in enumerate(gens) if is_r]

    pad_idx = (np.arange(PH) - HALO_L) % n
    xh = x.reshape(B, C, 2, n)[:, :, :, pad_idx]          # [B, C, 2, PH]

    ws = np.zeros((128, WSW), dtype=_NP_IN)
    for k, (j, _) in enumerate(rot + refl):
        for u in range(4):
            ws[32 * u:32 * (u + 1), 32 * k:32 * (k + 1)] = weight[j]

    def img(a, core):
        sl = a[core * BPC:(core + 1) * BPC]               # [64, C, 2, PH]
        out = np.empty((128, NQ, 2, PH), dtype=_NP_IN)
        for u in range(4):
            out[32 * u:32 * (u + 1)] = sl[u::4].transpose(1, 0, 2, 3)
        return np.ascontiguousarray(
            np.concatenate([ws, out.reshape(128, NQ * QW)], axis=1))

    return [img(xh, c) for c in range(N_CORES)]


# partition-block that holds slot u of quad q: supergroup quads are
# rotated (tile (u, (u+j)%4) with j the index within the group)
_GROUPS = [(0, 1), (2, 3, 4, 5), (6, 7, 8, 9), (10, 11), (12, 13), (14,),
           (15,)]
_GBASE = {q: g[0] for g in _GROUPS for q in g}
_VMAP = np.array([[(u + q - _GBASE[q]) % 4 for u in range(4)]
                  for q in range(NQ)])                       # [q, u] -> v


def _unscramble(outr):
    """outr[32*vmap[q,b%4]+d, (b>>2)*L + o] -> out shard [BPC, D, L]."""
    r = outr.astype(np.float32).reshape(4, D, NQ, L)    # [v, d, q, o]
    r = r.transpose(2, 0, 1, 3)                         # [q, v, d, o]
    r = r[np.arange(NQ)[:, None], _VMAP]                # [q, u, d, o]
    return np.ascontiguousarray(r.reshape(BPC, D, L))


def kernel(x, weight, perm, _trace=False):
    x = np.asarray(x, dtype=np.float32)
    weight = np.asarray(weight, dtype=np.float32)
    perm = np.asarray(perm, dtype=np.float32)

    gens = _derive_gens(perm)
    key = tuple(gens)
    if key not in _cache:
        _cache[key] = _build_program(gens)
    nc = _cache[key]

    axws = _host_images(x, weight, gens)
    in_maps = [{"axw": axws[c]} for c in range(N_CORES)]
    res = run_bass_kernel_spmd(nc, in_maps, core_ids=list(range(N_CORES)),
                               trace=_trace)
    out = np.concatenate([_unscramble(res.results[c]["outr"])
                          for c in range(N_CORES)], axis=0)
    if _trace:
        kernel.last_exec_time_ns = res.exec_time_ns
        kernel.last_results = res
    return out
